# revision 18
# baseline (speedup 1.0000x reference)
"""Trainium2 Bass kernel for nn_AGCRN_Model (gnn_message_passing).

Self-contained: call kernel(**inputs) with the full reference inputs.

Algorithm (reference simplifies because H==0 throughout):
  per layer L: pre = A_norm @ x @ Wcat_L + A_norm-agg(eattr @ Wecat_L) + b_L
               h   = (1-sigmoid(pre_R)) * tanh(pre_U)   [relu after layer 0]
Sharding: by destination node. Core c owns 1280 node slots (10 blocks x 128),
all 12 timesteps. Host relabels nodes (degree-balanced blocks), sorts edges by
owning (core, block), and bakes the full GCN norm rs[dst]*rs[src] into the
one-hot scatter matrices (so gather tables hold RAW x / h rows). Device:
dma_gather pulls per-edge rows from a bf16 node table; per 128-edge chunk a
PE matmul with the scatter matrix accumulates into PSUM per node block; edge
features aggregate as (S @ eaT) @ Wecat (8-wide matmuls) instead of
materializing per-edge embeddings; PE transposes + block-diag matmuls apply
Wcat; ACT does sigmoid/tanh. One AllGather shares the h table between layers
(its rank-major layout equals the slot numbering, so layer 1 reuses layer
0's gather indices).
"""
import sys

sys.path.insert(0, '/opt/trn_rl_repo')

import numpy as np
import ml_dtypes

import concourse.bass as bass
import concourse.mybir as mybir
from concourse import bacc, tile
from concourse.bass_utils import run_bass_kernel_spmd

N = 10000
E = 100000
T = 12
CIN = 16
CE = 8
HID = 32
NCORES = 8
NBLK = 10
NODES_PER_CORE = NBLK * 128          # 1280
N_PAD = NCORES * NODES_PER_CORE      # 10240

bf16 = ml_dtypes.bfloat16
DT = mybir.dt


def _to_bf16(a):
    return np.asarray(a).astype(bf16)


# ---------------------------------------------------------------- host prep
def _host_prep(x, edge_index, edge_attr,
               Wg0, Weg0, bg0, Wu0, Weu0, bu0,
               Wg1, Weg1, bg1, Wu1, Weu1, bu1):
    X = np.asarray(x, np.float32)[0]                  # [T, N, CIN]
    src = np.asarray(edge_index[0]).astype(np.int64)
    dst = np.asarray(edge_index[1]).astype(np.int64)
    ea = np.asarray(edge_attr, np.float32)            # [E, CE]

    deg = np.maximum(np.bincount(dst, minlength=N).astype(np.float32), 1.0)
    rs = 1.0 / np.sqrt(deg)

    # --- node -> (core, block, offset): snake-deal by degree, then bin-pack
    order = np.argsort(-deg, kind='stable')
    core_of = np.empty(N, np.int64)
    for i, n in enumerate(order):
        k = i % (2 * NCORES)
        core_of[n] = k if k < NCORES else 2 * NCORES - 1 - k
    perm_slot = np.empty(N, np.int64)
    for c in range(NCORES):
        nodes_c = np.where(core_of == c)[0]
        nodes_c = nodes_c[np.argsort(-deg[nodes_c], kind='stable')]
        blk_load = np.zeros(NBLK)
        blk_fill = np.zeros(NBLK, np.int64)
        for n in nodes_c:
            cand = np.where(blk_fill < 128)[0]
            b = cand[np.argmin(blk_load[cand])]
            perm_slot[n] = c * NODES_PER_CORE + b * 128 + blk_fill[b]
            blk_fill[b] += 1
            blk_load[b] += deg[n]
    slot_node = np.full(N_PAD, -1, np.int64)
    slot_node[perm_slot] = np.arange(N)
    valid = slot_node >= 0
    nvalid = slot_node[valid]

    # --- gather table for layer 0: [N_PAD, 256] bf16, (t, c) major, RAW x
    xt = np.zeros((N_PAD, 256), np.float32)
    xt[valid, :T * CIN] = np.transpose(X[:, nvalid, :], (1, 0, 2)).reshape(
        len(nvalid), T * CIN)
    x_table = _to_bf16(xt)

    # --- per-core edge structures
    dslot = perm_slot[dst]
    dcore = dslot // NODES_PER_CORE
    dblk = (dslot % NODES_PER_CORE) // 128
    doff = dslot % 128

    maxblk = 0
    core_edges = []
    for c in range(NCORES):
        per_blk = []
        for b in range(NBLK):
            es = np.where((dcore == c) & (dblk == b))[0]
            es = es[np.argsort(doff[es], kind='stable')]
            per_blk.append(es)
            maxblk = max(maxblk, len(es))
        core_edges.append(per_blk)
    C = int(np.ceil(maxblk / 128))
    E_blk = C * 128
    E_pad = NBLK * E_blk

    per_core = []
    for c in range(NCORES):
        idx = np.zeros(E_pad, np.int16)
        sc = np.zeros((NBLK * C, 128, 128), np.float32)   # (chunk, edge_row, node_off)
        eaTT = np.zeros((128, NBLK * C, CE), np.float32)  # (edge_row, chunk, ce)
        for b in range(NBLK):
            es = core_edges[c][b]
            k = len(es)
            idx[b * E_blk:b * E_blk + k] = perm_slot[src[es]].astype(np.int16)
            rows = np.arange(k)
            sc[b * C + rows // 128, rows % 128, doff[es]] = rs[dst[es]] * rs[src[es]]
            eaTT[rows % 128, b * C + rows // 128, :] = ea[es]
        idx_w = np.tile(idx.reshape(-1, 16).T, (8, 1)).copy()
        sc_dev = _to_bf16(np.transpose(sc, (1, 0, 2)))    # [128, NBLK*C, 128]
        per_core.append(dict(idx=idx_w, sc=sc_dev, eaTT=_to_bf16(eaTT)))

    # --- weights / consts
    Wg0, Wu0 = np.asarray(Wg0, np.float32), np.asarray(Wu0, np.float32)
    Wg1, Wu1 = np.asarray(Wg1, np.float32), np.asarray(Wu1, np.float32)
    Wcat0 = np.concatenate([Wg0[:CIN, HID:], Wu0[:CIN]], axis=1)      # [16, 64]
    Wcat1 = np.concatenate([Wg1[:HID, HID:], Wu1[:HID]], axis=1)      # [32, 64]
    w0_bd = np.zeros((96, 384), np.float32)      # 6 t-blocks of [16, 64]
    for tt in range(6):
        w0_bd[tt * 16:(tt + 1) * 16, tt * 64:(tt + 1) * 64] = Wcat0
    wcat0_rep = _to_bf16(w0_bd)
    w1_bd = np.zeros((96, 192), np.float32)      # 3 t-blocks of [32, 64]
    for tt in range(3):
        w1_bd[tt * 32:(tt + 1) * 32, tt * 64:(tt + 1) * 64] = Wcat1
    wcat1_rep = _to_bf16(w1_bd)
    Wecat_both = _to_bf16(np.concatenate(
        [np.asarray(Weg0, np.float32)[:, HID:], np.asarray(Weu0, np.float32),
         np.asarray(Weg1, np.float32)[:, HID:], np.asarray(Weu1, np.float32)],
        axis=1))                                                       # [8, 128]
    bcat0 = np.concatenate([np.asarray(bg0, np.float32)[HID:], np.asarray(bu0, np.float32)])
    bcat1 = np.concatenate([np.asarray(bg1, np.float32)[HID:], np.asarray(bu1, np.float32)])
    bb0 = np.tile(bcat0[None, :], (128, 1)).astype(np.float32)         # [128, 64]
    bb1 = np.tile(bcat1[None, :], (128, 1)).astype(np.float32)

    ident = _to_bf16(np.eye(128, dtype=np.float32))

    shared = dict(x_table=x_table, wcat0_rep=wcat0_rep, wcat1_rep=wcat1_rep,
                  wecat=Wecat_both, bb0=bb0, bb1=bb1, ident=ident)
    return shared, per_core, perm_slot, C


# ---------------------------------------------------------------- bass build
import os


def _build_nc(C):
    E_blk = C * 128
    E_pad = NBLK * E_blk
    nc = bacc.Bacc(None, target_bir_lowering=False, num_swdge_queues=4)

    x_table = nc.declare_dram_parameter("x_table", [N_PAD, 256], DT.bfloat16, isOutput=False)
    idx_d = nc.declare_dram_parameter("idx", [128, E_pad // 16], DT.int16, isOutput=False)
    sc_d = nc.declare_dram_parameter("sc", [128, NBLK * C, 128], DT.bfloat16, isOutput=False)
    eaTT_d = nc.declare_dram_parameter("eaTT", [128, NBLK * C, CE], DT.bfloat16, isOutput=False)
    wecat_d = nc.declare_dram_parameter("wecat", [CE, 128], DT.bfloat16, isOutput=False)
    w0_d = nc.declare_dram_parameter("wcat0_rep", [96, 384], DT.bfloat16, isOutput=False)
    w1_d = nc.declare_dram_parameter("wcat1_rep", [96, 192], DT.bfloat16, isOutput=False)
    bb0_d = nc.declare_dram_parameter("bb0", [128, 64], DT.float32, isOutput=False)
    bb1_d = nc.declare_dram_parameter("bb1", [128, 64], DT.float32, isOutput=False)
    ident_d = nc.declare_dram_parameter("ident", [128, 128], DT.bfloat16, isOutput=False)
    out_d = nc.declare_dram_parameter("out", [NODES_PER_CORE, T * HID], DT.float32, isOutput=True)

    # h exchanged in fp8 (e4m3), rows padded to 512B for gather alignment
    h_slice = nc.dram_tensor("h_slice", [NODES_PER_CORE, 512], DT.float8e4)
    h_table = nc.dram_tensor("h_table", [N_PAD, 512], DT.float8e4, addr_space="Shared")

    with tile.TileContext(nc) as tc:
        with (
            tc.tile_pool(name="const", bufs=1) as constp,
            tc.tile_pool(name="big", bufs=1) as bigp,
            tc.tile_pool(name="msg0", bufs=5) as msg0p,
            tc.tile_pool(name="msg1", bufs=5) as msg1p,
            tc.tile_pool(name="work", bufs=2) as workp,
            tc.tile_pool(name="psum_cat", bufs=2, space="PSUM") as pcatp,
            tc.tile_pool(name="psum_mix", bufs=2, space="PSUM") as pmixp,
            tc.tile_pool(name="psum_ew", bufs=1, space="PSUM") as pewp,
            tc.tile_pool(name="psum_xw", bufs=1, space="PSUM") as pxwp,
        ):
            # ---- constants; gather/ew inputs first so gathers start ASAP
            idx_sb = constp.tile([128, E_pad // 16], DT.int16)
            nc.sync.dma_start(idx_sb[:], idx_d[:])
            eaTT_sb = constp.tile([128, NBLK * C, CE], DT.bfloat16)
            nc.sync.dma_start(eaTT_sb[:], eaTT_d[:])
            wecat_sb = constp.tile([CE, 128], DT.bfloat16)
            nc.sync.dma_start(wecat_sb[:], wecat_d[:])
            sc_sb = bigp.tile([128, NBLK * C, 128], DT.bfloat16)
            for b in range(NBLK):
                nc.sync.dma_start(sc_sb[:, b * C:(b + 1) * C, :],
                                  sc_d[:, b * C:(b + 1) * C, :])
            w0_sb = constp.tile([96, 384], DT.bfloat16)
            nc.sync.dma_start(w0_sb[:], w0_d[:])
            w1_sb = constp.tile([96, 192], DT.bfloat16)
            nc.sync.dma_start(w1_sb[:], w1_d[:])
            bb0_sb = constp.tile([128, 64], DT.float32)
            nc.sync.dma_start(bb0_sb[:], bb0_d[:])
            bb1_sb = constp.tile([128, 64], DT.float32)
            nc.sync.dma_start(bb1_sb[:], bb1_d[:])
            ident_sb = constp.tile([128, 128], DT.bfloat16)
            nc.sync.dma_start(ident_sb[:], ident_d[:])

            ew1_sb = bigp.tile([128, NBLK, 64], DT.float32)

            NSUB = 2 if C % 2 == 0 else 1
            HB = E_blk // NSUB
            gq = [0]

            def gather(lidx, b, msgp_, table, melem):
                mdt = DT.bfloat16 if lidx == 0 else DT.float8e4
                msg = msgp_.tile([128, C, melem], mdt, tag=f"m{lidx}")
                for j in range(NSUB):
                    nc.gpsimd.dma_gather(
                        msg[:, j * (C // NSUB):(j + 1) * (C // NSUB), :], table[:],
                        idx_sb[:, (b * E_blk + j * HB) // 16:(b * E_blk + (j + 1) * HB) // 16],
                        HB, HB, melem, single_packet=False,
                        queue_num=gq[0] % 4)
                    gq[0] += 1
                return msg

            def compute_block(lidx, b, msg):
                cw = CIN if lidx == 0 else HID        # channels per t
                fa = T * cw                           # agg width (192 / 384)
                nhalf = fa // 96                      # transpose halves (2 / 4)
                if lidx == 1:
                    # cast the fp8 h rows back to bf16 for the PE
                    mcast = workp.tile([128, C, fa], DT.bfloat16, tag="mcast")
                    nc.vector.tensor_copy(mcast[:], msg[:, :, 0:fa])
                    msg = mcast
                pcat = pcatp.tile([128, fa], DT.float32, tag="pcat")
                if lidx == 0:
                    peagg = pewp.tile([128, CE], DT.float32, tag="peagg", bufs=1)
                for ch in range(C):
                    scl = sc_sb[:, b * C + ch, :]
                    nc.tensor.matmul(pcat[:], scl, msg[:, ch, 0:fa],
                                     start=(ch == 0), stop=(ch == C - 1))
                    if lidx == 0:
                        nc.tensor.matmul(peagg[:], scl,
                                         eaTT_sb[:, b * C + ch, :],
                                         start=(ch == 0), stop=(ch == C - 1))
                prebase = workp.tile([128, 64], DT.float32, tag="prebase")
                if lidx == 0:
                    # ew aggregate = (S @ eaT) @ Wecat_both
                    eagg_sb = workp.tile([128, CE], DT.bfloat16, tag="eagg")
                    nc.vector.tensor_copy(eagg_sb[:], peagg[:])
                    pT2 = pmixp.tile([CE, 128], DT.bfloat16, tag="pmix")
                    nc.tensor.transpose(pT2[:], eagg_sb[:], ident_sb[:])
                    eaggT = workp.tile([CE, 128], DT.bfloat16, tag="eaggT")
                    nc.vector.tensor_copy(eaggT[:], pT2[:])
                    pewb = pewp.tile([128, 128], DT.float32, tag="pewb", bufs=1)
                    nc.tensor.matmul(pewb[:], eaggT[:], wecat_sb[:],
                                     start=True, stop=True)
                    nc.vector.tensor_copy(ew1_sb[:, b, :], pewb[:, 64:128])
                    nc.vector.tensor_add(prebase[:], pewb[:, 0:64], bb0_sb[:])
                else:
                    nc.vector.tensor_add(prebase[:], ew1_sb[:, b, :], bb1_sb[:])
                agg_bf = workp.tile([128, fa], DT.bfloat16, tag="agg_bf")
                nc.vector.tensor_copy(agg_bf[:], pcat[:])
                aggT = workp.tile([96, nhalf, 128], DT.bfloat16, tag="aggT")
                for hh in range(nhalf):
                    pT = pmixp.tile([96, 128], DT.bfloat16, tag="pmix")
                    nc.tensor.transpose(pT[:], agg_bf[:, hh * 96:(hh + 1) * 96],
                                        ident_sb[:])
                    nc.vector.tensor_copy(aggT[:, hh, :], pT[:])
                pxw = pxwp.tile([128, 1024], DT.float32, tag="pxw")
                if lidx == 0:
                    nh, hstride, width, na, wsb = 2, 512, 384, 6, w0_sb
                else:
                    nh, hstride, width, na, wsb = 4, 256, 192, 3, w1_sb
                for hh in range(nh):
                    nc.tensor.matmul(
                        pxw[:, hh * hstride:hh * hstride + width],
                        aggT[:, hh, :], wsb[:], start=True, stop=True)
                xw_view = (pxw[:].rearrange("p (h x) -> p h x", h=nh)
                           [:, :, 0:width]
                           .rearrange("p h (a d) -> p h a d", d=64))
                pre = workp.tile([128, T, 64], DT.float32, tag="pre")
                nc.vector.tensor_add(
                    pre[:].rearrange("p (h a) d -> p h a d", h=nh),
                    xw_view,
                    prebase[:].unsqueeze(1).unsqueeze(1)
                    .broadcast_to((128, nh, na, 64)))
                oneR = workp.tile([128, T, 32], DT.float32, tag="oneR")
                nc.scalar.activation(oneR[:], pre[:, :, 0:32],
                                     mybir.ActivationFunctionType.Sigmoid,
                                     scale=-1.0)
                hc = workp.tile([128, T, 32], DT.float32, tag="hc")
                nc.scalar.activation(hc[:], pre[:, :, 32:64],
                                     mybir.ActivationFunctionType.Tanh)
                if lidx == 0:
                    # h = (1-R) * relu(HC), fp8 (raw; norms live in sc)
                    h_f8 = workp.tile([128, 512], DT.float8e4, tag="h_f8")
                    nc.vector.scalar_tensor_tensor(
                        h_f8[:, 0:T * HID].rearrange("p (t d) -> p t d", d=32),
                        hc[:], 0.0, oneR[:],
                        mybir.AluOpType.max, mybir.AluOpType.mult)
                    nc.sync.dma_start(h_slice[b * 128:(b + 1) * 128, :], h_f8[:])
                else:
                    o_sb = workp.tile([128, T * HID], DT.float32, tag="o_sb")
                    nc.vector.tensor_mul(
                        o_sb[:].rearrange("p (t d) -> p t d", d=32),
                        hc[:], oneR[:])
                    nc.sync.dma_start(out_d[b * 128:(b + 1) * 128, :], o_sb[:])

            # ---- layer 0
            msgs0 = {}
            for b in range(3):
                msgs0[b] = gather(0, b, msg0p, x_table, 256)
            for b in range(NBLK):
                compute_block(0, b, msgs0[b])
                if b + 3 < NBLK:
                    msgs0[b + 3] = gather(0, b + 3, msg0p, x_table, 256)

            # ---- h exchange (rank-major AG output == slot numbering)
            nc.gpsimd.collective_compute(
                "AllGather", mybir.AluOpType.bypass,
                replica_groups=[list(range(NCORES))],
                ins=[h_slice[:]], outs=[h_table[:]])

            # ---- layer 1 (reuses layer-0 gather indices)
            msgs1 = {}
            for b in range(NBLK):
                msgs1[b] = gather(1, b, msg1p, h_table, 512)
            for b in range(NBLK):
                compute_block(1, b, msgs1[b])

    nc.compile()
    return nc


_NC_CACHE = {}
_LAST_RESULT = None


def kernel(**inputs) -> np.ndarray:
    shared, per_core, perm_slot, C = _host_prep(**inputs)
    if C not in _NC_CACHE:
        _NC_CACHE[C] = _build_nc(C)
    nc = _NC_CACHE[C]
    in_maps = []
    for c in range(NCORES):
        m = dict(
            x_table=np.ascontiguousarray(shared['x_table']),
            idx=np.ascontiguousarray(per_core[c]['idx']),
            sc=np.ascontiguousarray(per_core[c]['sc']),
            eaTT=np.ascontiguousarray(per_core[c]['eaTT']),
            wecat=shared['wecat'], wcat0_rep=shared['wcat0_rep'],
            wcat1_rep=shared['wcat1_rep'], bb0=shared['bb0'], bb1=shared['bb1'],
            ident=shared['ident'],
        )
        in_maps.append(m)
    trace = bool(os.environ.get('KTRACE'))
    if trace:
        try:
            import ntff_shim  # registers the axon NTFF profile hook
        except Exception:
            pass
    res = run_bass_kernel_spmd(nc, in_maps, core_ids=list(range(NCORES)),
                               trace=trace)
    global _LAST_RESULT
    _LAST_RESULT = res
    out_pad = np.concatenate([res.results[c]["out"] for c in range(NCORES)], axis=0)
    out = out_pad[perm_slot].reshape(N, T, HID).transpose(1, 0, 2)
    return np.ascontiguousarray(out.astype(np.float32))


if __name__ == "__main__":
    pass
